# revision 1
# baseline (speedup 1.0000x reference)
"""Trainium2 Bass kernel for nn_LoRALinear (DoRA-style LoRA linear).

Reference math (per problem):
    base = x @ W^T
    lora = sc * (x @ A^T) @ B^T          (sc = 2.0)
    w_eff = W + sc * (B @ A)
    s = magnitude / ||w_eff||_row         (row norm over in_dim)
    out = base + (s - 1) * base + s * lora
        = s * (base + lora)
        = x @ (s[:, None] * w_eff)^T

So the whole op collapses to one dense matmul with a derived weight.

Strategy: data-parallel shard x over batch*seq across 8 cores; every core
redundantly derives w_eff^T (+ row norms + scale) on device from the small
replicated weights, then computes its x-shard's matmul in fp32r (FP22
multiplies, fp32 accumulate) on the PE array.

Per-core pipeline:
  setup:  BAT = (2A)^T-slices @ B^T  (PE, contraction over r=16 padded to 128)
          w_effT[k] = W^T[k] + BAT[k]            (DVE, in-place on W^T tile)
          sq = w_effT^2                           (ACT)
          norm2 = ones^T @ sq  (PE column-sum accumulated over k tiles)
          s = mag * rsqrt(norm2)  (ACT sqrt + DVE reciprocal + 2 Newton steps)
          s_rep = broadcast s to 128 partitions   (GPSIMD)
  main loop over 32 m-tiles (128 tokens each):
          DMA x tile [128, 1024]
          PE-transpose x into xT tiles (fp32r, 4 transposes packed per bank)
          ACT copy xT psum -> SBUF
          16 fp32r matmuls (8 k-tiles x 2 n-halves) accumulate in PSUM
          DVE multiply by s_rep (psum -> sbuf)
          DMA out tile [128, 1024]
"""

import os
import numpy as np
from contextlib import ExitStack

import concourse.bass as bass
import concourse.mybir as mybir
import concourse.tile as tile
from concourse import bacc
from concourse.bass import ts
from concourse.bass_utils import run_bass_kernel_spmd
from concourse.masks import make_identity

N_CORES = 8
B, S, D_IN, D_OUT, R = 4, 8192, 1024, 1024, 16
SCALING = 32.0 / 16.0
M_TOT = B * S                 # 32768 tokens
M_CORE = M_TOT // N_CORES     # 4096 tokens per core
P = 128
M_TILES = M_CORE // P         # 32
K_TILES = D_IN // P           # 8
NH = D_OUT // 512             # 2 n-halves of 512
F32 = mybir.dt.float32
F32R = mybir.dt.float32r


def _kernel_body(ctx: ExitStack, tc: "tile.TileContext", x, wT, a2p, bTp, mag, out):
    nc = tc.nc
    const_pool = ctx.enter_context(tc.tile_pool(name="const", bufs=1))
    w_pool = ctx.enter_context(tc.tile_pool(name="w", bufs=1))
    sq_pool = ctx.enter_context(tc.tile_pool(name="sq", bufs=2))
    x_pool = ctx.enter_context(tc.tile_pool(name="x", bufs=4))
    xt_pool = ctx.enter_context(tc.tile_pool(name="xt", bufs=4))
    o_pool = ctx.enter_context(tc.tile_pool(name="o", bufs=6))
    ps_tr = ctx.enter_context(tc.tile_pool(name="ps_tr", bufs=2, space="PSUM"))
    ps_out = ctx.enter_context(tc.tile_pool(name="ps_out", bufs=4, space="PSUM"))
    ps_norm = ctx.enter_context(tc.tile_pool(name="ps_norm", bufs=2, space="PSUM"))

    # ---- constants ----
    ident = const_pool.tile([P, P], F32)
    make_identity(nc, ident[:])
    ones_f = const_pool.tile([P, 1], F32)
    nc.vector.memset(ones_f[:], 1.0)
    ones = const_pool.tile([P, 1], F32R)
    nc.vector.tensor_copy(ones[:], ones_f[:])
    # walrus requires operands of fp32r matmuls to be PRODUCED as fp32r
    # (explicitly rounded), so stage through fp32 then round-copy on DVE.
    a2_f = const_pool.tile([P, D_IN], F32)
    nc.sync.dma_start(a2_f[:], a2p[:, :])
    a2_sb = const_pool.tile([P, D_IN], F32R)
    nc.vector.tensor_copy(a2_sb[:], a2_f[:])
    bT_f = const_pool.tile([P, D_OUT], F32)
    nc.sync.dma_start(bT_f[:], bTp[:, :])
    bT_sb = const_pool.tile([P, D_OUT], F32R)
    nc.vector.tensor_copy(bT_sb[:], bT_f[:])
    mag_sb = const_pool.tile([1, D_OUT], F32)
    nc.sync.dma_start(mag_sb[:], mag[:, :])

    # ---- derive w_effT = W^T + (2 B A)^T, tile by tile over k (d_in) ----
    wt_pool = ctx.enter_context(tc.tile_pool(name="wt", bufs=2))
    norm2_ps = [
        ps_norm.tile([1, 512], F32, tag="norm", name=f"norm2_{h}") for h in range(NH)
    ]
    weff = []
    for k in range(K_TILES):
        wt = wt_pool.tile([P, D_OUT], F32, tag="wt", name=f"wt{k}")
        nc.sync.dma_start(wt[:], wT[ts(k, P), :])
        weff_k = w_pool.tile([P, D_OUT], F32R, tag=f"weff{k}", name=f"weff{k}")
        for h in range(NH):
            bat = ps_out.tile([P, 512], F32, tag="out", name=f"bat{k}_{h}")
            nc.tensor.matmul(
                bat[:],
                lhsT=a2_sb[:, ts(k, P)],
                rhs=bT_sb[:, ts(h, 512)],
                start=True,
                stop=True,
            )
            # fp32 add, rounded to fp32r on write
            nc.vector.tensor_add(weff_k[:, ts(h, 512)], wt[:, ts(h, 512)], bat[:])
        # row-norm^2 contribution: column sums of squares via ones-matmul
        sqt = sq_pool.tile([P, D_OUT], F32R, tag="sq", name=f"sq{k}")
        nc.scalar.square(sqt[:], weff_k[:])
        for h in range(NH):
            nc.tensor.matmul(
                norm2_ps[h][:],
                lhsT=ones[:],
                rhs=sqt[:, ts(h, 512)],
                start=(k == 0),
                stop=(k == K_TILES - 1),
            )
        weff.append(weff_k)

    # ---- s = mag / sqrt(norm2), refined; broadcast to all partitions ----
    norm2_sb = const_pool.tile([1, D_OUT], F32)
    for h in range(NH):
        nc.scalar.copy(norm2_sb[:, ts(h, 512)], norm2_ps[h][:])
    # rsqrt(n) = exp(-0.5 * ln(n)), then one Newton step to kill LUT error
    lnn = const_pool.tile([1, D_OUT], F32)
    nc.scalar.activation(lnn[:], norm2_sb[:], mybir.ActivationFunctionType.Ln)
    y = const_pool.tile([1, D_OUT], F32)
    nc.scalar.activation(
        y[:], lnn[:], mybir.ActivationFunctionType.Exp, bias=0.0, scale=-0.5
    )
    t = const_pool.tile([1, D_OUT], F32)
    nc.vector.tensor_mul(t[:], y[:], y[:])     # Newton: y <- y*(1.5 - 0.5*n*y^2)
    nc.vector.tensor_mul(t[:], t[:], norm2_sb[:])
    nc.vector.tensor_scalar(
        t[:], t[:], -0.5, 1.5, mybir.AluOpType.mult, mybir.AluOpType.add
    )
    nc.vector.tensor_mul(y[:], y[:], t[:])
    s1 = const_pool.tile([1, D_OUT], F32)
    nc.vector.tensor_mul(s1[:], mag_sb[:], y[:])
    # broadcast s to all 128 partitions via a DRAM round trip with a
    # stride-0 partition read (partition_broadcast needs a ucode library
    # that is not loaded in this environment)
    dram_pool = ctx.enter_context(tc.tile_pool(name="dram", bufs=1, space="DRAM"))
    s_dram = dram_pool.tile([1, D_OUT], F32)
    nc.sync.dma_start(s_dram[:], s1[:])
    sd = s_dram[:]
    s_bcast_ap = bass.AP(tensor=sd.tensor, offset=sd.offset, ap=[[0, P], *sd.ap])
    s_rep = const_pool.tile([P, D_OUT], F32)
    nc.gpsimd.dma_start(out=s_rep[:], in_=s_bcast_ap)

    # ---- main loop over token tiles ----
    for m in range(M_TILES):
        x_sb = x_pool.tile([P, D_IN], F32, tag="x")
        nc.sync.dma_start(x_sb[:], x[ts(m, P), :])

        xt_sb = xt_pool.tile([P, D_IN], F32R, tag="xt")
        for g in range(2):  # 4 transposes packed into each psum bank
            ptr = ps_tr.tile([P, 512], F32, tag="tr")
            for j in range(4):
                k = 4 * g + j
                nc.tensor.transpose(
                    ptr[:, ts(j, P)],
                    x_sb[:, ts(k, P)],
                    ident[:],
                )
            # psum fp32 -> sbuf fp32r (rounding copy on ACT)
            nc.scalar.copy(xt_sb[:, ts(g, 512)], ptr[:])

        o_sb = o_pool.tile([P, D_OUT], F32, tag="o")
        # k-groups of 4 interleaved across the two n-halves: the first 8
        # matmuls depend only on transpose-group 0's copy, giving ACT ~1.9us
        # to land transpose-group 1's copy before it is needed
        psos = [ps_out.tile([P, 512], F32, tag="out", name=f"pso{h}") for h in range(NH)]
        for kg in range(2):
            for h in range(NH):
                for k in range(4 * kg, 4 * kg + 4):
                    nc.tensor.matmul(
                        psos[h][:],
                        lhsT=xt_sb[:, ts(k, P)],
                        rhs=weff[k][:, ts(h, 512)],
                        start=(k == 0),
                        stop=(k == K_TILES - 1),
                    )
        for h in range(NH):
            # plain drain (no s dependency) so psum slots recycle immediately;
            # the scale is applied in SBUF afterwards
            nc.scalar.copy(o_sb[:, ts(h, 512)], psos[h][:])
        nc.vector.tensor_mul(o_sb[:], o_sb[:], s_rep[:])
        nc.sync.dma_start(out[ts(m, P), :], o_sb[:])


def build_nc() -> "bass.Bass":
    nc = bacc.Bacc(
        "TRN2",
        target_bir_lowering=False,
        debug=False,
        num_devices=N_CORES,
    )
    x = nc.dram_tensor("x", [M_CORE, D_IN], F32, kind="ExternalInput").ap()
    wT = nc.dram_tensor("wT", [D_IN, D_OUT], F32, kind="ExternalInput").ap()
    a2p = nc.dram_tensor("a2p", [P, D_IN], F32, kind="ExternalInput").ap()
    bTp = nc.dram_tensor("bTp", [P, D_OUT], F32, kind="ExternalInput").ap()
    mag = nc.dram_tensor("mag", [1, D_OUT], F32, kind="ExternalInput").ap()
    out = nc.dram_tensor("out", [M_CORE, D_OUT], F32, kind="ExternalOutput").ap()

    with tile.TileContext(nc) as tc, ExitStack() as ctx:
        _kernel_body(ctx, tc, x, wT, a2p, bTp, mag, out)
    nc.compile()
    return nc


_NC_CACHE: list = []


def get_nc() -> "bass.Bass":
    if not _NC_CACHE:
        _NC_CACHE.append(build_nc())
    return _NC_CACHE[0]


def make_in_maps(x, weight, a_w, b_w, magnitude):
    xf = np.ascontiguousarray(x.reshape(M_TOT, D_IN).astype(np.float32, copy=False))
    wT = np.ascontiguousarray(weight.astype(np.float32, copy=False).T)
    a2p = np.zeros((P, D_IN), np.float32)
    a2p[:R] = SCALING * a_w
    bTp = np.zeros((P, D_OUT), np.float32)
    bTp[:R] = b_w.astype(np.float32, copy=False).T
    mag = np.ascontiguousarray(magnitude.astype(np.float32, copy=False))
    return [
        {
            "x": xf[i * M_CORE : (i + 1) * M_CORE],
            "wT": wT,
            "a2p": a2p,
            "bTp": bTp,
            "mag": mag,
        }
        for i in range(N_CORES)
    ]


def kernel(x, weight, a_w, b_w, magnitude):
    nc = get_nc()
    in_maps = make_in_maps(x, weight, a_w, b_w, magnitude)
    trace = os.environ.get("KERNEL_TRACE", "0") == "1"
    res = run_bass_kernel_spmd(nc, in_maps, list(range(N_CORES)), trace=trace)
    if trace:
        kernel.last_result = res
    outs = [res.results[i]["out"] for i in range(N_CORES)]
    return np.concatenate(outs, axis=0).reshape(B, S, D_OUT)



# revision 2
# speedup vs baseline: 1.2718x; 1.2718x over previous
"""Trainium2 Bass kernel for nn_LoRALinear (DoRA-style LoRA linear).

Reference math (per problem):
    base = x @ W^T
    lora = sc * (x @ A^T) @ B^T          (sc = 2.0)
    w_eff = W + sc * (B @ A)
    s = magnitude / ||w_eff||_row         (row norm over in_dim)
    out = base + (s - 1) * base + s * lora
        = s * (base + lora)
        = x @ (s[:, None] * w_eff)^T

So the whole op collapses to one dense matmul with a derived weight.

Strategy: data-parallel shard x over batch*seq across 8 cores; every core
redundantly derives w_eff (bf16) + row norms + scale s on device from the
small replicated weights, then computes its x-shard's matmul in bf16
(fp32 accumulate) on the PE array.

The host pre-stages layout only: x is transposed to d-major tiles and
rounded to bf16 (as is W^T / A / B^T), so the device spends zero PE cycles
transposing and streams the matmul at 1 column/cycle with half-size
weight loads. All arithmetic (w_eff derivation, norms, rsqrt, scaling,
the big matmul) runs on device.

Per-core pipeline:
  setup:  BAT = (2A)^T-slices @ B^T  (PE, contraction over r=16 padded 128)
          w_effT[k] = W^T[k] + BAT[k]  -> bf16   (DVE)
          sq = w_effT^2 (ACT), norm2 = ones^T @ sq (PE, accumulated)
          s = mag * rsqrt(norm2) (ACT/DVE + Newton), broadcast to 128
          partitions via DRAM stride-0 read
  main loop over 8 chunks of 512 tokens (one [128, 4096] bf16 DMA each):
          4 t-tiles per chunk; per t-tile 16 bf16 matmuls (8 k x 2 halves)
          accumulate in PSUM; DVE multiplies by s_rep (psum -> sbuf);
          DMA out tile [128, 1024] fp32.
"""

import os
import numpy as np
from contextlib import ExitStack

import ml_dtypes

import concourse.bass as bass
import concourse.mybir as mybir
import concourse.tile as tile
from concourse import bacc
from concourse.bass import ts
from concourse.bass_utils import run_bass_kernel_spmd

N_CORES = 8
B, S, D_IN, D_OUT, R = 4, 8192, 1024, 1024, 16
SCALING = 32.0 / 16.0
M_TOT = B * S                 # 32768 tokens
M_CORE = M_TOT // N_CORES     # 4096 tokens per core
P = 128
CHUNK_T = 512                 # tokens per DMA chunk
N_CHUNKS = M_CORE // CHUNK_T  # 8
TPC = CHUNK_T // P            # 4 t-tiles per chunk
K_TILES = D_IN // P           # 8
NH = D_OUT // 512             # 2 n-halves of 512
F32 = mybir.dt.float32
BF16 = mybir.dt.bfloat16
NPBF16 = ml_dtypes.bfloat16


def _kernel_body(ctx: ExitStack, tc: "tile.TileContext", xp, wT, a2p, bTp, mag, out):
    nc = tc.nc
    const_pool = ctx.enter_context(tc.tile_pool(name="const", bufs=1))
    wt_pool = ctx.enter_context(tc.tile_pool(name="wt", bufs=2))
    w_pool = ctx.enter_context(tc.tile_pool(name="w", bufs=1))
    sq_pool = ctx.enter_context(tc.tile_pool(name="sq", bufs=2))
    x_pool = ctx.enter_context(tc.tile_pool(name="x", bufs=2))
    o_pool = ctx.enter_context(tc.tile_pool(name="o", bufs=4))
    ps_pool = ctx.enter_context(tc.tile_pool(name="ps", bufs=6, space="PSUM"))
    ps_norm = ctx.enter_context(tc.tile_pool(name="ps_norm", bufs=1, space="PSUM"))
    dram_pool = ctx.enter_context(tc.tile_pool(name="dram", bufs=1, space="DRAM"))

    # ---- constants / small inputs ----
    ones_f = const_pool.tile([P, 1], F32)
    nc.vector.memset(ones_f[:], 1.0)
    ones = const_pool.tile([P, 1], BF16)
    nc.vector.tensor_copy(ones[:], ones_f[:])
    a2_sb = const_pool.tile([P, D_IN], BF16)
    nc.sync.dma_start(a2_sb[:], a2p[:, :])
    bT_sb = const_pool.tile([P, D_OUT], BF16)
    nc.sync.dma_start(bT_sb[:], bTp[:, :])
    mag_sb = const_pool.tile([1, D_OUT], F32)
    nc.sync.dma_start(mag_sb[:], mag[:, :])

    # ---- derive w_effT = W^T + (2 B A)^T in bf16, tile by tile over k ----
    weff = []
    for k in range(K_TILES):
        wt = wt_pool.tile([P, D_OUT], BF16, tag="wt", name=f"wt{k}")
        nc.sync.dma_start(wt[:], wT[ts(k, P), :])
        weff_k = w_pool.tile([P, D_OUT], BF16, tag=f"weff{k}", name=f"weff{k}")
        for h in range(NH):
            bat = ps_pool.tile([P, 512], F32, tag="mm", name=f"bat{k}_{h}")
            nc.tensor.matmul(
                bat[:],
                lhsT=a2_sb[:, ts(k, P)],
                rhs=bT_sb[:, ts(h, 512)],
                start=True,
                stop=True,
            )
            # fp32 add, rounded to bf16 on write
            nc.vector.tensor_add(weff_k[:, ts(h, 512)], wt[:, ts(h, 512)], bat[:])
        weff.append(weff_k)

    # ---- row-norm^2 via ones-matmul over squared tiles ----
    norm2_ps = [
        ps_norm.tile([1, 512], F32, tag=f"norm{h}", name=f"norm2_{h}") for h in range(NH)
    ]
    for k in range(K_TILES):
        sqt = sq_pool.tile([P, D_OUT], BF16, tag="sq", name=f"sq{k}")
        nc.scalar.square(sqt[:], weff[k][:])
        for h in range(NH):
            nc.tensor.matmul(
                norm2_ps[h][:],
                lhsT=ones[:],
                rhs=sqt[:, ts(h, 512)],
                start=(k == 0),
                stop=(k == K_TILES - 1),
            )

    # ---- s = mag / sqrt(norm2), refined; broadcast to all partitions ----
    norm2_sb = const_pool.tile([1, D_OUT], F32)
    for h in range(NH):
        nc.scalar.copy(norm2_sb[:, ts(h, 512)], norm2_ps[h][:])
    # rsqrt(n) = exp(-0.5 * ln(n)), then one Newton step to kill LUT error
    lnn = const_pool.tile([1, D_OUT], F32)
    nc.scalar.activation(lnn[:], norm2_sb[:], mybir.ActivationFunctionType.Ln)
    y = const_pool.tile([1, D_OUT], F32)
    nc.scalar.activation(
        y[:], lnn[:], mybir.ActivationFunctionType.Exp, bias=0.0, scale=-0.5
    )
    t = const_pool.tile([1, D_OUT], F32)
    nc.vector.tensor_mul(t[:], y[:], y[:])     # Newton: y <- y*(1.5 - 0.5*n*y^2)
    nc.vector.tensor_mul(t[:], t[:], norm2_sb[:])
    nc.vector.tensor_scalar(
        t[:], t[:], -0.5, 1.5, mybir.AluOpType.mult, mybir.AluOpType.add
    )
    nc.vector.tensor_mul(y[:], y[:], t[:])
    s1 = const_pool.tile([1, D_OUT], F32)
    nc.vector.tensor_mul(s1[:], mag_sb[:], y[:])
    # broadcast s to all 128 partitions via a DRAM round trip with a
    # stride-0 partition read
    s_dram = dram_pool.tile([1, D_OUT], F32)
    nc.sync.dma_start(s_dram[:], s1[:])
    sd = s_dram[:]
    s_bcast_ap = bass.AP(tensor=sd.tensor, offset=sd.offset, ap=[[0, P], *sd.ap])
    s_rep = const_pool.tile([P, D_OUT], F32)
    nc.gpsimd.dma_start(out=s_rep[:], in_=s_bcast_ap)

    # ---- main loop over 512-token chunks ----
    # xp rows c*128+p hold x^T data: xp[c*128+p, k*512+t] = x[c*512+t, k*128+p]
    for c in range(N_CHUNKS):
        xch = x_pool.tile([P, K_TILES * CHUNK_T], BF16, tag="x", name=f"x{c}")
        nc.sync.dma_start(xch[:], xp[ts(c, P), :])
        for mt in range(TPC):
            pss = [
                ps_pool.tile([P, 512], F32, tag="mm", name=f"pso{c}_{mt}_{h}")
                for h in range(NH)
            ]
            for k in range(K_TILES):
                lhsT = xch[:, k * CHUNK_T + mt * P : k * CHUNK_T + (mt + 1) * P]
                for h in range(NH):
                    nc.tensor.matmul(
                        pss[h][:],
                        lhsT=lhsT,
                        rhs=weff[k][:, ts(h, 512)],
                        start=(k == 0),
                        stop=(k == K_TILES - 1),
                    )
            o_sb = o_pool.tile([P, D_OUT], F32, tag="o")
            for h in range(NH):
                nc.vector.tensor_mul(o_sb[:, ts(h, 512)], pss[h][:], s_rep[:, ts(h, 512)])
            nc.sync.dma_start(out[ts(c * TPC + mt, P), :], o_sb[:])


def build_nc() -> "bass.Bass":
    nc = bacc.Bacc(
        "TRN2",
        target_bir_lowering=False,
        debug=False,
        num_devices=N_CORES,
    )
    xp = nc.dram_tensor("xp", [M_CORE // CHUNK_T * P, K_TILES * CHUNK_T], BF16,
                        kind="ExternalInput").ap()
    wT = nc.dram_tensor("wT", [D_IN, D_OUT], BF16, kind="ExternalInput").ap()
    a2p = nc.dram_tensor("a2p", [P, D_IN], BF16, kind="ExternalInput").ap()
    bTp = nc.dram_tensor("bTp", [P, D_OUT], BF16, kind="ExternalInput").ap()
    mag = nc.dram_tensor("mag", [1, D_OUT], F32, kind="ExternalInput").ap()
    out = nc.dram_tensor("out", [M_CORE, D_OUT], F32, kind="ExternalOutput").ap()

    with tile.TileContext(nc) as tc, ExitStack() as ctx:
        _kernel_body(ctx, tc, xp, wT, a2p, bTp, mag, out)
    nc.compile()
    return nc


_NC_CACHE: list = []


def get_nc() -> "bass.Bass":
    if not _NC_CACHE:
        _NC_CACHE.append(build_nc())
    return _NC_CACHE[0]


def make_in_maps(x, weight, a_w, b_w, magnitude):
    xf = x.reshape(M_TOT, D_IN).astype(NPBF16)
    # per-core d-major chunk layout: [chunks, d-in-k-tile, k, t]
    # xp[c*128+p, k*512+t] = x_core[c*512+t, k*128+p]
    xcs = xf.reshape(N_CORES, N_CHUNKS, CHUNK_T, K_TILES, P)
    xcs = np.ascontiguousarray(xcs.transpose(0, 1, 4, 3, 2))
    xcs = xcs.reshape(N_CORES, N_CHUNKS * P, K_TILES * CHUNK_T)
    wTb = np.ascontiguousarray(weight.astype(np.float32, copy=False).T).astype(NPBF16)
    a2p = np.zeros((P, D_IN), NPBF16)
    a2p[:R] = (SCALING * a_w).astype(NPBF16)
    bTp = np.zeros((P, D_OUT), NPBF16)
    bTp[:R] = b_w.astype(np.float32, copy=False).T.astype(NPBF16)
    mag = np.ascontiguousarray(magnitude.astype(np.float32, copy=False))
    return [
        {
            "xp": xcs[i],
            "wT": wTb,
            "a2p": a2p,
            "bTp": bTp,
            "mag": mag,
        }
        for i in range(N_CORES)
    ]


def kernel(x, weight, a_w, b_w, magnitude):
    nc = get_nc()
    in_maps = make_in_maps(x, weight, a_w, b_w, magnitude)
    trace = os.environ.get("KERNEL_TRACE", "0") == "1"
    res = run_bass_kernel_spmd(nc, in_maps, list(range(N_CORES)), trace=trace)
    if trace:
        kernel.last_result = res
    outs = [res.results[i]["out"] for i in range(N_CORES)]
    return np.concatenate(outs, axis=0).reshape(B, S, D_OUT)


# revision 5
# speedup vs baseline: 1.3036x; 1.0250x over previous
"""Trainium2 Bass kernel for nn_LoRALinear (DoRA-style LoRA linear).

Reference math (per problem):
    base = x @ W^T
    lora = sc * (x @ A^T) @ B^T          (sc = 2.0)
    w_eff = W + sc * (B @ A)
    s = magnitude / ||w_eff||_row         (row norm over in_dim)
    out = base + (s - 1) * base + s * lora
        = s * (base + lora)
        = x @ (s[:, None] * w_eff)^T

So the whole op collapses to one dense matmul with a derived weight.

Strategy: data-parallel shard x over batch*seq across 8 cores; every core
redundantly derives w_eff (bf16) + row norms + scale s on device from the
small replicated weights, then computes its x-shard's matmul in bf16
(fp32 accumulate) on the PE array.

The host pre-stages layout only: x is transposed to d-major tiles and
rounded to bf16 (as is W^T / A / B^T), so the device spends zero PE cycles
transposing and streams the matmul at 1 column/cycle with half-size
weight loads. All arithmetic (w_eff derivation, norms, rsqrt, scaling,
the big matmul) runs on device.

Per-core pipeline:
  setup:  BAT = (2A)^T-slices @ B^T  (PE, contraction over r=16 padded 128)
          w_effT[k] = W^T[k] + BAT[k]  -> bf16   (DVE)
          sq = w_effT^2 (ACT), norm2 = ones^T @ sq (PE, accumulated)
          s = mag * rsqrt(norm2) (ACT/DVE + Newton), broadcast to 128
          partitions via DRAM stride-0 read
  main loop over 8 chunks of 512 tokens (one [128, 4096] bf16 DMA each):
          4 t-tiles per chunk; per t-tile 16 bf16 matmuls (8 k x 2 halves)
          accumulate in PSUM; DVE multiplies by s_rep (psum -> sbuf);
          DMA out tile [128, 1024] fp32.
"""

import os
import numpy as np
from contextlib import ExitStack

import ml_dtypes

import concourse.bass as bass
import concourse.mybir as mybir
import concourse.tile as tile
from concourse import bacc
from concourse.bass import ts
from concourse.bass_utils import run_bass_kernel_spmd

N_CORES = 8
B, S, D_IN, D_OUT, R = 4, 8192, 1024, 1024, 16
SCALING = 32.0 / 16.0
M_TOT = B * S                 # 32768 tokens
M_CORE = M_TOT // N_CORES     # 4096 tokens per core
P = 128
CHUNK_T = 512                 # tokens per DMA chunk
N_CHUNKS = M_CORE // CHUNK_T  # 8
TPC = CHUNK_T // P            # 4 t-tiles per chunk
K_TILES = D_IN // P           # 8
NH = D_OUT // 512             # 2 n-halves of 512
F32 = mybir.dt.float32
BF16 = mybir.dt.bfloat16
NPBF16 = ml_dtypes.bfloat16


def _kernel_body(ctx: ExitStack, tc: "tile.TileContext", xp, wT, abp, mag, out):
    nc = tc.nc
    const_pool = ctx.enter_context(tc.tile_pool(name="const", bufs=1))
    wt_pool = ctx.enter_context(tc.tile_pool(name="wt", bufs=2))
    w_pool = ctx.enter_context(tc.tile_pool(name="w", bufs=1))
    sq_pool = ctx.enter_context(tc.tile_pool(name="sq", bufs=2))
    x_pool = ctx.enter_context(tc.tile_pool(name="x", bufs=2))
    o_pool = ctx.enter_context(tc.tile_pool(name="o", bufs=6))
    ps_pool = ctx.enter_context(tc.tile_pool(name="ps", bufs=6, space="PSUM"))
    ps_norm = ctx.enter_context(tc.tile_pool(name="ps_norm", bufs=1, space="PSUM"))
    dram_pool = ctx.enter_context(tc.tile_pool(name="dram", bufs=1, space="DRAM"))

    # ---- constants / small inputs ----
    ones_f = const_pool.tile([P, 1], F32)
    nc.vector.memset(ones_f[:], 1.0)
    ones = const_pool.tile([P, 1], BF16)
    nc.vector.tensor_copy(ones[:], ones_f[:])
    ab_sb = const_pool.tile([P, 2 * D_IN], BF16)
    nc.sync.dma_start(ab_sb[:], abp[:, :])
    a2_sb = ab_sb[:, :D_IN]
    bT_sb = ab_sb[:, D_IN:]
    mag_sb = const_pool.tile([1, D_OUT], F32)
    nc.scalar.dma_start(mag_sb[:], mag[:, :])

    # ---- derive w_effT = W^T + (2 B A)^T in bf16, tile by tile over k;
    # squares + norm accumulation interleave so s is ready early ----
    norm2_ps = [
        ps_norm.tile([1, 512], F32, tag=f"norm{h}", name=f"norm2_{h}") for h in range(NH)
    ]
    weff = []
    for k in range(K_TILES):
        wt = wt_pool.tile([P, D_OUT], BF16, tag="wt", name=f"wt{k}")
        nc.sync.dma_start(wt[:], wT[ts(k, P), :])
        weff_k = w_pool.tile([P, D_OUT], BF16, tag=f"weff{k}", name=f"weff{k}")
        for h in range(NH):
            bat = ps_pool.tile([P, 512], F32, tag="mm", name=f"bat{k}_{h}")
            nc.tensor.matmul(
                bat[:],
                lhsT=a2_sb[:, ts(k, P)],
                rhs=bT_sb[:, ts(h, 512)],
                start=True,
                stop=True,
            )
            # fp32 add, rounded to bf16 on write
            nc.vector.tensor_add(weff_k[:, ts(h, 512)], wt[:, ts(h, 512)], bat[:])
        sqt = sq_pool.tile([P, D_OUT], BF16, tag="sq", name=f"sq{k}")
        nc.scalar.square(sqt[:], weff_k[:])
        for h in range(NH):
            nc.tensor.matmul(
                norm2_ps[h][:],
                lhsT=ones[:],
                rhs=sqt[:, ts(h, 512)],
                start=(k == 0),
                stop=(k == K_TILES - 1),
            )
        weff.append(weff_k)

    # ---- s = mag / sqrt(norm2), refined; broadcast to all partitions ----
    norm2_sb = const_pool.tile([1, D_OUT], F32)
    for h in range(NH):
        nc.scalar.copy(norm2_sb[:, ts(h, 512)], norm2_ps[h][:])
    # rsqrt(n) = exp(-0.5 * ln(n)), then one Newton step to kill LUT error
    lnn = const_pool.tile([1, D_OUT], F32)
    nc.scalar.activation(lnn[:], norm2_sb[:], mybir.ActivationFunctionType.Ln)
    y = const_pool.tile([1, D_OUT], F32)
    nc.scalar.activation(
        y[:], lnn[:], mybir.ActivationFunctionType.Exp, bias=0.0, scale=-0.5
    )
    t = const_pool.tile([1, D_OUT], F32)
    nc.vector.tensor_mul(t[:], y[:], y[:])     # Newton: y <- y*(1.5 - 0.5*n*y^2)
    nc.vector.tensor_mul(t[:], t[:], norm2_sb[:])
    nc.vector.tensor_scalar(
        t[:], t[:], -0.5, 1.5, mybir.AluOpType.mult, mybir.AluOpType.add
    )
    nc.vector.tensor_mul(y[:], y[:], t[:])
    s1 = const_pool.tile([1, D_OUT], F32)
    nc.vector.tensor_mul(s1[:], mag_sb[:], y[:])
    # broadcast s to all 128 partitions via a DRAM round trip with a
    # stride-0 partition read
    s_dram = dram_pool.tile([1, D_OUT], F32)
    nc.sync.dma_start(s_dram[:], s1[:])
    sd = s_dram[:]
    s_bcast_ap = bass.AP(tensor=sd.tensor, offset=sd.offset, ap=[[0, P], *sd.ap])
    s_rep = const_pool.tile([P, D_OUT], F32)
    nc.gpsimd.dma_start(out=s_rep[:], in_=s_bcast_ap)

    # ---- main loop over 512-token chunks ----
    # xp rows c*128+p hold x^T data: xp[c*128+p, k*512+t] = x[c*512+t, k*128+p]
    for c in range(N_CHUNKS):
        xch = x_pool.tile([P, K_TILES * CHUNK_T], BF16, tag="x", name=f"x{c}")
        nc.sync.dma_start(xch[:], xp[ts(c, P), :])
        for mt in range(TPC):
            pss = [
                ps_pool.tile([P, 512], F32, tag="mm", name=f"pso{c}_{mt}_{h}")
                for h in range(NH)
            ]
            for k in range(K_TILES):
                lhsT = xch[:, k * CHUNK_T + mt * P : k * CHUNK_T + (mt + 1) * P]
                for h in range(NH):
                    nc.tensor.matmul(
                        pss[h][:],
                        lhsT=lhsT,
                        rhs=weff[k][:, ts(h, 512)],
                        start=(k == 0),
                        stop=(k == K_TILES - 1),
                    )
            o_sb = o_pool.tile([P, D_OUT], F32, tag="o")
            for h in range(NH):
                # plain drain (no s dependency) so psum slots recycle
                # immediately; the scale is applied in SBUF afterwards
                nc.scalar.copy(o_sb[:, ts(h, 512)], pss[h][:])
            nc.vector.tensor_mul(o_sb[:], o_sb[:], s_rep[:])
            nc.sync.dma_start(out[ts(c * TPC + mt, P), :], o_sb[:])


def build_nc() -> "bass.Bass":
    nc = bacc.Bacc(
        "TRN2",
        target_bir_lowering=False,
        debug=False,
        num_devices=N_CORES,
    )
    xp = nc.dram_tensor("xp", [M_CORE // CHUNK_T * P, K_TILES * CHUNK_T], BF16,
                        kind="ExternalInput").ap()
    wT = nc.dram_tensor("wT", [D_IN, D_OUT], BF16, kind="ExternalInput").ap()
    abp = nc.dram_tensor("abp", [P, 2 * D_IN], BF16, kind="ExternalInput").ap()
    mag = nc.dram_tensor("mag", [1, D_OUT], F32, kind="ExternalInput").ap()
    out = nc.dram_tensor("out", [M_CORE, D_OUT], F32, kind="ExternalOutput").ap()

    with tile.TileContext(nc) as tc, ExitStack() as ctx:
        _kernel_body(ctx, tc, xp, wT, abp, mag, out)
    nc.compile()
    return nc


_NC_CACHE: list = []


def get_nc() -> "bass.Bass":
    if not _NC_CACHE:
        _NC_CACHE.append(build_nc())
    return _NC_CACHE[0]


def make_in_maps(x, weight, a_w, b_w, magnitude):
    xf = x.reshape(M_TOT, D_IN).astype(NPBF16)
    # per-core d-major chunk layout: [chunks, d-in-k-tile, k, t]
    # xp[c*128+p, k*512+t] = x_core[c*512+t, k*128+p]
    xcs = xf.reshape(N_CORES, N_CHUNKS, CHUNK_T, K_TILES, P)
    xcs = np.ascontiguousarray(xcs.transpose(0, 1, 4, 3, 2))
    xcs = xcs.reshape(N_CORES, N_CHUNKS * P, K_TILES * CHUNK_T)
    wTb = np.ascontiguousarray(weight.astype(np.float32, copy=False).T).astype(NPBF16)
    abp = np.zeros((P, 2 * D_IN), NPBF16)
    abp[:R, :D_IN] = (SCALING * a_w).astype(NPBF16)
    abp[:R, D_IN:] = b_w.astype(np.float32, copy=False).T.astype(NPBF16)
    mag = np.ascontiguousarray(magnitude.astype(np.float32, copy=False))
    return [
        {
            "xp": xcs[i],
            "wT": wTb,
            "abp": abp,
            "mag": mag,
        }
        for i in range(N_CORES)
    ]


def kernel(x, weight, a_w, b_w, magnitude):
    nc = get_nc()
    in_maps = make_in_maps(x, weight, a_w, b_w, magnitude)
    trace = os.environ.get("KERNEL_TRACE", "0") == "1"
    res = run_bass_kernel_spmd(nc, in_maps, list(range(N_CORES)), trace=trace)
    if trace:
        kernel.last_result = res
    outs = [res.results[i]["out"] for i in range(N_CORES)]
    return np.concatenate(outs, axis=0).reshape(B, S, D_OUT)
